# revision 12
# baseline (speedup 1.0000x reference)
"""Trainium2 Bass kernel for nn_MultiHeadClassifier.

  logits[b, c] = sum_{(g,l): label_ids[g,l]==c} group_probs[b,g] *
                 (features[b] @ W[g,l] + b[g,l])

Data-parallel over batch (8 cores, 4096 rows each). Per core:
  * Host prep: pack the G*L=1024 head outputs into 8 chunks of exactly
    128 rows with (a) no class split across chunks and (b) a BALANCED
    group profile — every chunk holds exactly 8 rows of each of the 16
    groups at the same row positions (row r -> group r//8).  (a) makes
    each chunk's scatter ONE matmul (S_j [128,128] 0/1, zero-padded,
    stationary; batch moving); (b) lets all 8 chunks share one
    [128, 512] probs tile per b-tile.
  * GEMM1 (PE, fp8 e4m3 DoubleRow): X and W are quantized at scales 8
    and 64 with their residuals re-quantized at the SAME scales, and
    the product rebuilt as X8@W8 + X8@dW8 + dX8@W8 (the dX@dW term is
    ~1e-6 and dropped) — six K=256 DoubleRow matmuls at 0.5 cyc/row
    replace four fp16 K=128 matmuls, cutting PE cycles 25% and energy
    (less DVFS throttle).  Everything downstream runs at 512x scale;
    the host divides the output by 512 (exact, power of two).
    Measured end-to-end rel-err 9.0e-4 vs the fp32 reference.
  * Fused (DVE): wtj = (pg + 512*bias_j) * ptx via scalar_tensor_tensor.
  * Scatter (PE, fp16): one [128, 512] matmul per chunk, interleaved
    into the next tile's GEMM1 stream; the last tile's scatter is
    woven into its own GEMM1 stream to compress the tail.
  * Drain (ACT): per-chunk strip PSUM -> fp16 into a per-tile [128,
    4096] SBUF tile; two half out-DMAs per tile alternating sync/scalar
    queues.  PE warm-up matmuls during the gate DMA absorb the p-state
    ramp; the gate rides the earliest (gpsimd software-DGE) queue.
Output strips are fp16 at 512x; host maps strip rows back to class ids
and descales.
"""
import os
import sys
import numpy as np
import ml_dtypes

for _p in ("/opt/trn_rl_repo",):
    if _p not in sys.path:
        sys.path.append(_p)

import concourse.bass as bass  # noqa: E402
import concourse.tile as tile  # noqa: E402
from concourse import bacc, mybir, bass_utils  # noqa: E402
from contextlib import ExitStack  # noqa: E402

F32 = mybir.dt.float32
F16 = mybir.dt.float16
F8 = mybir.dt.float8e4
NP8 = ml_dtypes.float8_e4m3fn
DR = mybir.MatmulPerfMode.DoubleRow

B, F, G, L, C = 32768, 512, 16, 64, 1000
NCORE = 8
BC = B // NCORE          # 4096 batch rows per core
NT = BC // 512           # 8 b-tiles of 512
KF = F // 128            # 4 feature chunks
NCH = 8                  # 8 chunks of 128 head-outputs (exact, no pad)
GPC = 128 // G           # rows per group per chunk (8)
NWARM = 42               # PE p-state warm-up matmuls during the gate DMA
PW = 512 + NCH           # shared probs tile width (+8 bias cols)
SX, SW = 8.0, 64.0       # fp8 quantization scales for X and W
SXW = SX * SW            # PSUM scale (512)

LAST_EXEC_NS = None


def _host_prep(W, b, label_ids):
    """Pack classes whole into 8 chunks of exactly 128 rows with every
    chunk holding exactly GPC rows of each group (row r <-> group r//GPC
    for ALL chunks)."""
    lab = np.asarray(label_ids).reshape(-1).astype(np.int64)
    GL = lab.shape[0]
    Wflat = np.asarray(W, dtype=np.float32).reshape(GL, F)
    bflat = np.asarray(b, dtype=np.float32).reshape(GL)

    rows_of = {}
    for gl, c in enumerate(lab):
        rows_of.setdefault(int(c), []).append(gl)
    cvec = {}
    for c, rows in rows_of.items():
        v = np.zeros(G, dtype=np.int64)
        for gl in rows:
            v[gl // L] += 1
        cvec[c] = v
    target = np.full(G, GPC, dtype=np.int64)
    loads = [np.zeros(G, dtype=np.int64) for _ in range(NCH)]
    bins = [[] for _ in range(NCH)]
    order = sorted(rows_of, key=lambda c: (-len(rows_of[c]),
                                           tuple(-cvec[c])))
    for c in order:
        v = cvec[c]
        best, bestslack = None, -1
        for i in range(NCH):
            if np.all(loads[i] + v <= target):
                slack = int((target - loads[i]).sum())
                if slack > bestslack:
                    best, bestslack = i, slack
        if best is None:
            raise RuntimeError("balanced class packing failed")
        loads[best] += v
        bins[best].append(c)
    assert all(np.all(ld == target) for ld in loads)

    ws = [len(bn) for bn in bins]
    WT = np.zeros((F, NCH * 128), dtype=np.float32)
    biasT = np.zeros((128, NCH), dtype=np.float32)
    SS = np.zeros((128, NCH * 128), dtype=np.float16)  # per-chunk padded
    for j, bn in enumerate(bins):
        slot = [g * GPC for g in range(G)]   # next free row per group
        for m, c in enumerate(bn):
            for gl in rows_of[c]:
                g = gl // L
                r = slot[g]
                slot[g] += 1
                WT[:, j * 128 + r] = Wflat[gl]
                biasT[r, j] = bflat[gl]
                SS[r, j * 128 + m] = 1.0
        assert slot == [(g + 1) * GPC for g in range(G)]
    return dict(WT=WT, biasT=biasT, SS=SS, ws=ws, bins=bins)


def _build_program():
    nc = bacc.Bacc("TRN2", target_bir_lowering=False, debug=False,
                   num_devices=NCORE)
    # fp8 X pair-blocks: row block bi holds tiles (2bi+1, 2bi+2);
    # dim1 = 4*half + k.  t0 and t7 ride in their own tensors.
    x8_d = nc.dram_tensor("x8", [3 * 128, 2 * KF, 512], F8,
                          kind="ExternalInput").ap()
    dx8_d = nc.dram_tensor("dx8", [3 * 128, 2 * KF, 512], F8,
                           kind="ExternalInput").ap()
    x08_d = nc.dram_tensor("x08", [128, KF, 512], F8,
                           kind="ExternalInput").ap()
    dx08_d = nc.dram_tensor("dx08", [128, KF, 512], F8,
                            kind="ExternalInput").ap()
    x78_d = nc.dram_tensor("x78", [128, KF, 512], F8,
                           kind="ExternalInput").ap()
    dx78_d = nc.dram_tensor("dx78", [128, KF, 512], F8,
                            kind="ExternalInput").ap()
    # fp8 W blocks per chunk: dim2 0..3 = W8 k, 4..7 = dW8 k
    w8_d = nc.dram_tensor("w8", [128, NCH, 2 * KF, 128], F8,
                          kind="ExternalInput").ap()
    # shared probs per b-tile (row r -> group r//GPC) + 8 bias*512 cols
    ptx_d = nc.dram_tensor("ptx", [NT * 128, PW], F16,
                           kind="ExternalInput").ap()
    s_d = nc.dram_tensor("s", [128, NCH * 128], F16,
                         kind="ExternalInput").ap()
    # out: block (t, j) at cols (t*8+j)*512; rows 0..w_j are chunk j's
    # class strips (transposed, class-major) at 512x scale.
    out_d = nc.dram_tensor("logits", [128, NT * NCH * 512], F16,
                           kind="ExternalOutput").ap()

    with tile.TileContext(nc) as tc, ExitStack() as ctx:
        const = ctx.enter_context(tc.tile_pool(name="const", bufs=1))
        psG = ctx.enter_context(tc.tile_pool(name="psG", bufs=5, space="PSUM"))
        psL = ctx.enter_context(tc.tile_pool(name="psL", bufs=3, space="PSUM"))
        sbW = ctx.enter_context(tc.tile_pool(name="sbW", bufs=18))
        sbO = ctx.enter_context(tc.tile_pool(name="sbO", bufs=4))

        # gate on the earliest (gpsimd) queue: W-j0 then X8-t0; dX8-t0
        # and W-j1 ride the scalar queue in parallel.
        wj8 = [None] * NCH
        for j in range(NCH):
            wj8[j] = const.tile([128, 2 * KF, 128], F8, name=f"wj8{j}",
                                tag=f"wj8{j}")
        nc.gpsimd.dma_start(wj8[0][:], w8_d[:, 0, :, :])
        wgx8 = const.tile([128, KF, 512], F8, name="wgx8", tag="wgx8")
        nc.gpsimd.dma_start(wgx8[:], x08_d[:])
        for j in range(2, NCH):
            nc.gpsimd.dma_start(wj8[j][:], w8_d[:, j, :, :])
        nc.scalar.dma_start(wj8[1][:], w8_d[:, 1, :, :])
        wgdx8 = const.tile([128, KF, 512], F8, name="wgdx8", tag="wgdx8")
        nc.scalar.dma_start(wgdx8[:], dx08_d[:])

        # PE warm-up: ramp the tensor-engine p-state while the gate DMA
        # streams.  Source is a memset tile; results are never read.
        warm = const.tile([128, 384], F16, name="warm", tag="warm")
        nc.vector.memset(warm[:], 0.0)
        for i in range(NWARM):
            wps = psL.tile([128, 512], F32, name="pl", tag="pl")
            nc.tensor.matmul(wps[:, 0:256], warm[:, 0:128], warm[:, 128:384],
                             start=True, stop=True)

        # probs tiles: t0 first on scalar, rest follow there
        ptxs = []
        for t in range(NT):
            t_ = const.tile([128, PW], F16, name=f"ptx{t}", tag=f"ptx{t}")
            nc.scalar.dma_start(t_[:], ptx_d[t * 128:(t + 1) * 128, :])
            ptxs.append(t_)

        # sync queue: scatter matrix + X/dX pair blocks + t7
        ss = const.tile([128, NCH * 128], F16, name="ss", tag="ss")
        nc.sync.dma_start(ss[:], s_d[:])
        xp8 = [None] * 3
        dxp8 = [None] * 3
        for bi in range(3):
            xp8[bi] = const.tile([128, 2 * KF, 512], F8, name=f"xp8{bi}",
                                 tag=f"xp8{bi}")
            nc.sync.dma_start(xp8[bi][:], x8_d[bi * 128:(bi + 1) * 128, :, :])
            dxp8[bi] = const.tile([128, 2 * KF, 512], F8, name=f"dxp8{bi}",
                                  tag=f"dxp8{bi}")
            nc.sync.dma_start(dxp8[bi][:],
                              dx8_d[bi * 128:(bi + 1) * 128, :, :])
        x78 = const.tile([128, KF, 512], F8, name="x78", tag="x78")
        nc.sync.dma_start(x78[:], x78_d[:])
        dx78 = const.tile([128, KF, 512], F8, name="dx78", tag="dx78")
        nc.sync.dma_start(dx78[:], dx78_d[:])

        def x8_ap(t, kp):          # k-pair slice of the X8 tile for t
            if t == 0:
                return wgx8[:, 2 * kp:2 * kp + 2, :]
            if t == NT - 1:
                return x78[:, 2 * kp:2 * kp + 2, :]
            half = 0 if t % 2 == 1 else 4
            return xp8[(t - 1) // 2][:, half + 2 * kp:half + 2 * kp + 2, :]

        def dx8_ap(t, kp):
            if t == 0:
                return wgdx8[:, 2 * kp:2 * kp + 2, :]
            if t == NT - 1:
                return dx78[:, 2 * kp:2 * kp + 2, :]
            half = 0 if t % 2 == 1 else 4
            return dxp8[(t - 1) // 2][:, half + 2 * kp:half + 2 * kp + 2, :]

        wtbuf = [[None] * NCH for _ in range(2)]

        def scatter_one(tt, j, ob, deng):
            pl = psL.tile([128, 512], F32, name="pl", tag="pl")
            nc.tensor.matmul(pl[:], ss[:, j * 128:(j + 1) * 128],
                             wtbuf[tt % 2][j][:], start=True, stop=True)
            if deng is nc.vector:
                deng.tensor_copy(ob[:, j * 512:(j + 1) * 512], pl[:])
            else:
                deng.activation(ob[:, j * 512:(j + 1) * 512], pl[:],
                                mybir.ActivationFunctionType.Copy,
                                bias=0.0, scale=1.0)

        ob_prev = ob_last = None
        for t in range(NT):
            if t >= 1:
                ob_prev = sbO.tile([128, NCH * 512], F16, name="obt",
                                   tag="obt")
            if t == NT - 1:
                ob_last = sbO.tile([128, NCH * 512], F16, name="obt",
                                   tag="obt")
            for j in range(NCH):
                pg = psG.tile([128, 512], F32, name="pg", tag="pg")
                # X8@W8 + X8@dW8 + dX8@W8, six K=256 DoubleRow matmuls
                nc.tensor.matmul(pg[:], wj8[j][:, 0:2, :], x8_ap(t, 0),
                                 start=True, stop=False, perf_mode=DR)
                nc.tensor.matmul(pg[:], wj8[j][:, 2:4, :], x8_ap(t, 1),
                                 start=False, stop=False, perf_mode=DR)
                nc.tensor.matmul(pg[:], wj8[j][:, 4:6, :], x8_ap(t, 0),
                                 start=False, stop=False, perf_mode=DR)
                nc.tensor.matmul(pg[:], wj8[j][:, 6:8, :], x8_ap(t, 1),
                                 start=False, stop=False, perf_mode=DR)
                nc.tensor.matmul(pg[:], wj8[j][:, 0:2, :], dx8_ap(t, 0),
                                 start=False, stop=False, perf_mode=DR)
                nc.tensor.matmul(pg[:], wj8[j][:, 2:4, :], dx8_ap(t, 1),
                                 start=False, stop=True, perf_mode=DR)
                wtj = sbW.tile([128, 512], F16, name="wtj", tag="wtj")
                nc.vector.scalar_tensor_tensor(
                    wtj[:], pg[:], ptxs[t][:, 512 + j:513 + j],
                    ptxs[t][:, 0:512],
                    op0=mybir.AluOpType.add, op1=mybir.AluOpType.mult)
                wtbuf[t % 2][j] = wtj
                if t >= 1:
                    scatter_one(t - 1, j, ob_prev, nc.scalar)
                if t == NT - 1 and j >= 2:
                    # weave the last tile's scatter into its own GEMM1
                    scatter_one(t, j - 2, ob_last,
                                nc.vector if (j - 2) % 2 else nc.scalar)
            if t >= 1:
                # two half DMAs per tile, queues alternating by parity
                col = (t - 1) * NCH * 512
                oeng = nc.sync if t % 2 == 0 else nc.scalar
                oeng.dma_start(out_d[:, col:col + 2048], ob_prev[:, 0:2048])
                oeng.dma_start(out_d[:, col + 2048:col + 4096],
                               ob_prev[:, 2048:4096])
        # tail: last tile's chunks 6,7 + staggered quarter out-DMAs
        col = (NT - 1) * NCH * 512
        nc.sync.dma_start(out_d[:, col:col + 2048], ob_last[:, 0:2048])
        scatter_one(NT - 1, 6, ob_last, nc.scalar)
        nc.scalar.dma_start(out_d[:, col + 2048:col + 3072],
                            ob_last[:, 2048:3072])
        scatter_one(NT - 1, 7, ob_last, nc.vector)
        nc.gpsimd.dma_start(out_d[:, col + 3072:col + 3584],
                            ob_last[:, 3072:3584])
        nc.sync.dma_start(out_d[:, col + 3584:col + 4096],
                          ob_last[:, 3584:4096])
    nc.finalize()
    return nc


def kernel(features, group_probs, W, b, label_ids):
    global LAST_EXEC_NS
    features = np.asarray(features, dtype=np.float32)
    group_probs = np.asarray(group_probs, dtype=np.float32)
    prep = _host_prep(W, b, label_ids)
    ws, bins = prep["ws"], prep["bins"]
    nc = _build_program()

    XT = features.T.astype(np.float32)                        # [F, B]
    PT = group_probs.T.astype(np.float16)                     # [G, B]
    # fp8 quantization at matched scales; residuals at the same scales
    XTs = XT * SX
    XT8 = XTs.astype(NP8)
    dXT8 = (XTs - XT8.astype(np.float32)).astype(NP8)
    WTs = prep["WT"] * SW                                     # [F, 1024]
    WT8 = WTs.astype(NP8)
    dWT8 = (WTs - WT8.astype(np.float32)).astype(NP8)
    # W blocks: w8[p, j, k, m] = WT8[k*128+p, j*128+m]; dim2 4..7 = dW8
    w8 = np.empty((128, NCH, 2 * KF, 128), dtype=NP8)
    for j in range(NCH):
        for k in range(KF):
            w8[:, j, k, :] = WT8[k * 128:(k + 1) * 128,
                                 j * 128:(j + 1) * 128]
            w8[:, j, KF + k, :] = dWT8[k * 128:(k + 1) * 128,
                                       j * 128:(j + 1) * 128]
    bias16 = (prep["biasT"] * SXW).astype(np.float16)         # [128, NCH]
    in_maps = []
    for c in range(NCORE):
        def xpack(XTq):
            # [t, p, k, cc] = XTq[k*128+p, t*512+cc]
            xc = XTq[:, c * BC:(c + 1) * BC].reshape(KF, 128, NT, 512)
            return np.ascontiguousarray(xc.transpose(2, 1, 0, 3))
        xf = xpack(XT8)                                       # [NT,128,4,512]
        dxf = xpack(dXT8)
        # pair blocks (t1,t2),(t3,t4),(t5,t6): dim1 = 4*half + k
        x8 = np.empty((3 * 128, 2 * KF, 512), dtype=NP8)
        dx8 = np.empty((3 * 128, 2 * KF, 512), dtype=NP8)
        for bi in range(3):
            x8[bi * 128:(bi + 1) * 128, 0:KF] = xf[2 * bi + 1]
            x8[bi * 128:(bi + 1) * 128, KF:] = xf[2 * bi + 2]
            dx8[bi * 128:(bi + 1) * 128, 0:KF] = dxf[2 * bi + 1]
            dx8[bi * 128:(bi + 1) * 128, KF:] = dxf[2 * bi + 2]
        # shared probs tile: row r -> group r//GPC, plus bias*512 cols
        ptc = PT[:, c * BC:(c + 1) * BC].reshape(G, NT, 512)  # [16, 8, 512]
        ptx = np.empty((NT, 128, PW), dtype=np.float16)
        ptx[:, :, 0:512] = np.repeat(ptc, GPC, axis=0).transpose(1, 0, 2)
        ptx[:, :, 512:PW] = bias16[None, :, :]
        in_maps.append({
            "x8": x8, "dx8": dx8,
            "x08": xf[0], "dx08": dxf[0],
            "x78": xf[7], "dx78": dxf[7],
            "w8": w8,
            "ptx": np.ascontiguousarray(ptx.reshape(NT * 128, PW)),
            "s": prep["SS"],
        })

    trace = bool(os.environ.get("BASS_TRACE"))
    if trace:
        bass_utils.upload_artifacts = lambda d: "local://skipped"
    try:
        res = bass_utils.run_bass_kernel_spmd(nc, in_maps,
                                              core_ids=list(range(NCORE)))
    except Exception:
        # transient NRT device errors have been observed; one retry
        res = bass_utils.run_bass_kernel_spmd(nc, in_maps,
                                              core_ids=list(range(NCORE)))
    if trace:
        LAST_EXEC_NS = res.exec_time_ns
        if res.exec_time_ns is not None:
            print(f"HW exec time: {res.exec_time_ns} ns")

    out = np.zeros((B, C), dtype=np.float32)
    inv = np.float32(1.0 / SXW)
    for c in range(NCORE):
        o2 = res.results[c]["logits"]                          # [128, 32768]
        r0 = c * BC
        for t in range(NT):
            for j in range(NCH):
                col = (t * NCH + j) * 512
                strip = o2[0:ws[j], col:col + 512]             # [w_j, 512]
                out[r0 + t * 512:r0 + (t + 1) * 512, bins[j]] = \
                    strip.T.astype(np.float32) * inv
    return out


# revision 13
# speedup vs baseline: 1.3466x; 1.3466x over previous
"""Trainium2 Bass kernel for nn_MultiHeadClassifier.

  logits[b, c] = sum_{(g,l): label_ids[g,l]==c} group_probs[b,g] *
                 (features[b] @ W[g,l] + b[g,l])

Data-parallel over batch (8 cores, 4096 rows each). Per core:
  * Host prep: pack the G*L=1024 head outputs into 8 chunks of exactly
    128 rows with (a) no class split across chunks and (b) a BALANCED
    group profile — every chunk holds exactly 8 rows of each of the 16
    groups at the same row positions (row r -> group r//8).  (a) makes
    each chunk's scatter ONE matmul (S_j [128,128] 0/1, zero-padded,
    stationary; batch moving); (b) lets all 8 chunks share one
    [128, 512] probs tile per b-tile.
  * GEMM1 (PE, fp16): pg[gl, b] = W^T.T @ X^T per (chunk, b-tile),
    4 accumulating K=128 matmuls.  The gate DMAs ride the two earliest
    queues (gpsimd software-DGE + scalar) and PE warm-up matmuls absorb
    the p-state/power ramp; W streams in per-chunk 128KB DMAs just
    ahead of consumption.  (fp8 DoubleRow was tried and is NOT faster:
    the PE column rate, ~216ns per 512-col matmul under the sustained
    power throttle, is identical for fp8-DR and fp16 — K-depth is free,
    so the 6-matmul compensated-fp8 scheme loses to 4 fp16 matmuls.)
  * Fused (DVE): wtj = (pg + bias_j) * ptx via scalar_tensor_tensor.
  * Scatter (PE, fp16): one [128, 512] matmul per chunk, interleaved
    into the next tile's GEMM1 stream; the last tile's scatter is
    woven into its own GEMM1 stream to compress the tail.
  * Drain (ACT): per-chunk strip PSUM -> fp16 into a per-tile [128,
    4096] SBUF tile; half out-DMAs issued mid-iteration (after strip 3)
    and at iteration end, queues alternating by tile parity; quarter
    DMAs on the final strips.
Output strips are fp16; host maps strip rows back to class ids.
"""
import os
import sys
import numpy as np

for _p in ("/opt/trn_rl_repo",):
    if _p not in sys.path:
        sys.path.append(_p)

import concourse.bass as bass  # noqa: E402
import concourse.tile as tile  # noqa: E402
from concourse import bacc, mybir, bass_utils  # noqa: E402
from contextlib import ExitStack  # noqa: E402

F32 = mybir.dt.float32
F16 = mybir.dt.float16

B, F, G, L, C = 32768, 512, 16, 64, 1000
NCORE = 8
BC = B // NCORE          # 4096 batch rows per core
NT = BC // 512           # 8 b-tiles of 512
KF = F // 128            # 4 feature chunks
NCH = 8                  # 8 chunks of 128 head-outputs (exact, no pad)
GPC = 128 // G           # rows per group per chunk (8)
NWARM = 44               # PE p-state warm-up matmuls during the gate DMA
PW = 512 + NCH           # shared probs tile width (+8 bias cols)

LAST_EXEC_NS = None


def _host_prep(W, b, label_ids):
    """Pack classes whole into 8 chunks of exactly 128 rows with every
    chunk holding exactly GPC rows of each group (row r <-> group r//GPC
    for ALL chunks)."""
    lab = np.asarray(label_ids).reshape(-1).astype(np.int64)
    GL = lab.shape[0]
    Wflat = np.asarray(W, dtype=np.float32).reshape(GL, F)
    bflat = np.asarray(b, dtype=np.float32).reshape(GL)

    rows_of = {}
    for gl, c in enumerate(lab):
        rows_of.setdefault(int(c), []).append(gl)
    cvec = {}
    for c, rows in rows_of.items():
        v = np.zeros(G, dtype=np.int64)
        for gl in rows:
            v[gl // L] += 1
        cvec[c] = v
    target = np.full(G, GPC, dtype=np.int64)
    loads = [np.zeros(G, dtype=np.int64) for _ in range(NCH)]
    bins = [[] for _ in range(NCH)]
    order = sorted(rows_of, key=lambda c: (-len(rows_of[c]),
                                           tuple(-cvec[c])))
    for c in order:
        v = cvec[c]
        best, bestslack = None, -1
        for i in range(NCH):
            if np.all(loads[i] + v <= target):
                slack = int((target - loads[i]).sum())
                if slack > bestslack:
                    best, bestslack = i, slack
        if best is None:
            raise RuntimeError("balanced class packing failed")
        loads[best] += v
        bins[best].append(c)
    assert all(np.all(ld == target) for ld in loads)

    ws = [len(bn) for bn in bins]
    WT = np.zeros((F, NCH * 128), dtype=np.float16)
    biasT = np.zeros((128, NCH), dtype=np.float32)
    SS = np.zeros((128, NCH * 128), dtype=np.float16)  # per-chunk padded
    for j, bn in enumerate(bins):
        slot = [g * GPC for g in range(G)]   # next free row per group
        for m, c in enumerate(bn):
            for gl in rows_of[c]:
                g = gl // L
                r = slot[g]
                slot[g] += 1
                WT[:, j * 128 + r] = Wflat[gl]
                biasT[r, j] = bflat[gl]
                SS[r, j * 128 + m] = 1.0
        assert slot == [(g + 1) * GPC for g in range(G)]
    return dict(WT=WT, biasT=biasT, SS=SS, ws=ws, bins=bins)


def _build_program():
    nc = bacc.Bacc("TRN2", target_bir_lowering=False, debug=False,
                   num_devices=NCORE)
    # xk: k-interleaved X^T packed by t-pair — row blocks hold
    # (t1,t2), (t3,t4), (t5,t6), (t7, pad). t0 rides in the gate.
    xk_d = nc.dram_tensor("xk", [(NT // 2) * 128, 2 * KF * 512], F16,
                          kind="ExternalInput").ap()
    # shared probs per b-tile (row r -> group r//GPC) + 8 bias cols
    ptx_d = nc.dram_tensor("ptx", [NT * 128, PW], F16,
                           kind="ExternalInput").ap()
    # wg: X^T t0 (k-interleaved, 2048 cols) + chunk-0's W (512 cols)
    wg_d = nc.dram_tensor("wg", [128, KF * 512 + KF * 128], F16,
                          kind="ExternalInput").ap()
    # wrest: W chunks 1..7, j-major: col (j-1)*512 + k*128
    wrest_d = nc.dram_tensor("wrest", [128, (NCH - 1) * KF * 128], F16,
                             kind="ExternalInput").ap()
    s_d = nc.dram_tensor("s", [128, NCH * 128], F16,
                         kind="ExternalInput").ap()
    # out: block (t, j) at cols (t*8+j)*512; rows 0..w_j are chunk j's
    # class strips (transposed, class-major)
    out_d = nc.dram_tensor("logits", [128, NT * NCH * 512], F16,
                           kind="ExternalOutput").ap()

    with tile.TileContext(nc) as tc, ExitStack() as ctx:
        const = ctx.enter_context(tc.tile_pool(name="const", bufs=1))
        psG = ctx.enter_context(tc.tile_pool(name="psG", bufs=5, space="PSUM"))
        psL = ctx.enter_context(tc.tile_pool(name="psL", bufs=3, space="PSUM"))
        sbW = ctx.enter_context(tc.tile_pool(name="sbW", bufs=18))
        sbO = ctx.enter_context(tc.tile_pool(name="sbO", bufs=4))

        # gate chain split across the two earliest queues: gpsimd takes
        # W-j0 + X-t0-k01, scalar takes X-t0-k23 (+ wr1, ptx0) — the
        # first matmul gates on 384KB instead of 640KB.
        wgw = const.tile([128, KF * 128], F16, name="wgw", tag="wgw")
        nc.gpsimd.dma_start(wgw[:], wg_d[:, KF * 512:])
        wgx01 = const.tile([128, 1024], F16, name="wgx01", tag="wgx01")
        nc.gpsimd.dma_start(wgx01[:], wg_d[:, 0:1024])
        wgx23 = const.tile([128, 1024], F16, name="wgx23", tag="wgx23")
        nc.scalar.dma_start(wgx23[:], wg_d[:, 1024:2048])
        wr = [None] * NCH
        for j in range(1, NCH):
            wr[j] = const.tile([128, KF * 128], F16, name=f"wr{j}",
                               tag=f"wr{j}")
        nc.scalar.dma_start(wr[1][:], wrest_d[:, 0:512])
        for j in range(2, NCH):
            nc.gpsimd.dma_start(wr[j][:], wrest_d[:, (j - 1) * 512:j * 512])

        # PE warm-up: ramp the tensor-engine p-state while the gate DMA
        # streams.  Source is a memset tile; results are never read.
        warm = const.tile([128, 384], F16, name="warm", tag="warm")
        nc.vector.memset(warm[:], 0.0)
        for i in range(NWARM):
            wps = psL.tile([128, 512], F32, name="pl", tag="pl")
            nc.tensor.matmul(wps[:, 0:256], warm[:, 0:128], warm[:, 128:384],
                             start=True, stop=True)

        # probs tiles (tiny): t0 first on scalar, rest follow there
        ptxs = []
        for t in range(NT):
            t_ = const.tile([128, PW], F16, name=f"ptx{t}", tag=f"ptx{t}")
            nc.scalar.dma_start(t_[:], ptx_d[t * 128:(t + 1) * 128, :])
            ptxs.append(t_)

        # sync queue: scatter matrix + remaining X pairs
        ss = const.tile([128, NCH * 128], F16, name="ss", tag="ss")
        nc.sync.dma_start(ss[:], s_d[:])
        xtile = [None] * NT     # (tile, col_base) per t; t0 in wgx01/23
        xp0 = const.tile([128, 4096], F16, name="xp0", tag="xp0")
        nc.sync.dma_start(xp0[:], xk_d[0:128, :])
        xtile[1] = (xp0, 0)
        xtile[2] = (xp0, 2048)
        for bi in range(1, 3):
            t_ = const.tile([128, 4096], F16, name=f"xp{bi}", tag=f"xp{bi}")
            nc.sync.dma_start(t_[:], xk_d[bi * 128:(bi + 1) * 128, :])
            xtile[2 * bi + 1] = (t_, 0)
            xtile[2 * bi + 2] = (t_, 2048)
        x7 = const.tile([128, 2048], F16, name="x7", tag="x7")
        nc.sync.dma_start(x7[:], xk_d[3 * 128:4 * 128, 0:2048])
        xtile[7] = (x7, 0)

        def w_ap(j, k):
            if j == 0:
                return wgw[:, k * 128:(k + 1) * 128]
            return wr[j][:, k * 128:(k + 1) * 128]

        def x_ap(t, k):
            if t == 0:
                if k < 2:
                    return wgx01[:, k * 512:(k + 1) * 512]
                return wgx23[:, (k - 2) * 512:(k - 1) * 512]
            xt_, xb = xtile[t]
            return xt_[:, xb + k * 512:xb + (k + 1) * 512]

        wtbuf = [[None] * NCH for _ in range(2)]

        def scatter_one(tt, j, ob, deng):
            pl = psL.tile([128, 512], F32, name="pl", tag="pl")
            nc.tensor.matmul(pl[:], ss[:, j * 128:(j + 1) * 128],
                             wtbuf[tt % 2][j][:], start=True, stop=True)
            if deng is nc.vector:
                deng.tensor_copy(ob[:, j * 512:(j + 1) * 512], pl[:])
            else:
                deng.activation(ob[:, j * 512:(j + 1) * 512], pl[:],
                                mybir.ActivationFunctionType.Copy,
                                bias=0.0, scale=1.0)

        ob_prev = ob_last = None
        for t in range(NT):
            if t >= 1:
                ob_prev = sbO.tile([128, NCH * 512], F16, name="obt",
                                   tag="obt")
            if t == NT - 1:
                ob_last = sbO.tile([128, NCH * 512], F16, name="obt",
                                   tag="obt")
            oeng = nc.sync if t % 2 == 0 else nc.scalar
            for j in range(NCH):
                pg = psG.tile([128, 512], F32, name="pg", tag="pg")
                for k in range(KF):
                    nc.tensor.matmul(pg[:], w_ap(j, k), x_ap(t, k),
                                     start=(k == 0), stop=(k == KF - 1))
                wtj = sbW.tile([128, 512], F16, name="wtj", tag="wtj")
                nc.vector.scalar_tensor_tensor(
                    wtj[:], pg[:], ptxs[t][:, 512 + j:513 + j],
                    ptxs[t][:, 0:512],
                    op0=mybir.AluOpType.add, op1=mybir.AluOpType.mult)
                wtbuf[t % 2][j] = wtj
                if t >= 1:
                    scatter_one(t - 1, j, ob_prev, nc.scalar)
                    if j == 3 or j == 7:
                        # half out-DMA as soon as its strips are drained
                        col = (t - 1) * NCH * 512 + (j - 3) * 512
                        oeng.dma_start(out_d[:, col:col + 2048],
                                       ob_prev[:, (j - 3) * 512:
                                               (j + 1) * 512])
                if t == NT - 1 and j >= 2:
                    # weave the last tile's scatter into its own GEMM1
                    scatter_one(t, j - 2, ob_last,
                                nc.vector if (j - 2) % 2 else nc.scalar)
        # tail: last tile's chunks 6,7 + staggered quarter out-DMAs
        col = (NT - 1) * NCH * 512
        nc.sync.dma_start(out_d[:, col:col + 2048], ob_last[:, 0:2048])
        scatter_one(NT - 1, 6, ob_last, nc.scalar)
        nc.scalar.dma_start(out_d[:, col + 2048:col + 3072],
                            ob_last[:, 2048:3072])
        scatter_one(NT - 1, 7, ob_last, nc.vector)
        nc.gpsimd.dma_start(out_d[:, col + 3072:col + 3584],
                            ob_last[:, 3072:3584])
        nc.sync.dma_start(out_d[:, col + 3584:col + 4096],
                          ob_last[:, 3584:4096])
    nc.finalize()
    return nc


def kernel(features, group_probs, W, b, label_ids):
    global LAST_EXEC_NS
    features = np.asarray(features, dtype=np.float32)
    group_probs = np.asarray(group_probs, dtype=np.float32)
    prep = _host_prep(W, b, label_ids)
    ws, bins = prep["ws"], prep["bins"]
    nc = _build_program()

    XT = features.T.astype(np.float16)                        # [F, B]
    PT = group_probs.T.astype(np.float16)                     # [G, B]
    WTf = prep["WT"]                                          # [F, 1024]
    # j-major W blocks, col j*512 + k*128 = WTf[k*128.., j*128..]
    wj = np.empty((128, NCH * KF * 128), dtype=np.float16)
    for j in range(NCH):
        for k in range(KF):
            wj[:, j * 512 + k * 128:j * 512 + (k + 1) * 128] = \
                WTf[k * 128:(k + 1) * 128, j * 128:(j + 1) * 128]
    bias16 = prep["biasT"].astype(np.float16)                 # [128, NCH]
    in_maps = []
    for c in range(NCORE):
        # k-interleaved X^T: xflat[t, p, k*512+cc] = XT[k*128+p, t*512+cc]
        xc = XT[:, c * BC:(c + 1) * BC].reshape(KF, 128, NT, 512)
        xflat = xc.transpose(2, 1, 0, 3).reshape(NT, 128, KF * 512)
        # wg = X t0 + W chunk 0; wrest = W chunks 1..7
        wgc = np.concatenate([xflat[0], wj[:, 0:512]], axis=1)
        wrestc = np.ascontiguousarray(wj[:, 512:])
        # xk row-blocks: (t1,t2), (t3,t4), (t5,t6), (t7, zero-pad)
        xk = np.zeros(((NT // 2) * 128, 2 * KF * 512), dtype=np.float16)
        for bi in range(3):
            xk[bi * 128:(bi + 1) * 128, :2048] = xflat[2 * bi + 1]
            xk[bi * 128:(bi + 1) * 128, 2048:] = xflat[2 * bi + 2]
        xk[3 * 128:4 * 128, :2048] = xflat[7]
        xk = np.ascontiguousarray(xk)
        # shared probs tile: row r -> group r//GPC, plus bias cols
        ptc = PT[:, c * BC:(c + 1) * BC].reshape(G, NT, 512)  # [16, 8, 512]
        ptx = np.empty((NT, 128, PW), dtype=np.float16)
        ptx[:, :, 0:512] = np.repeat(ptc, GPC, axis=0).transpose(1, 0, 2)
        ptx[:, :, 512:PW] = bias16[None, :, :]
        in_maps.append({
            "xk": xk,
            "ptx": np.ascontiguousarray(ptx.reshape(NT * 128, PW)),
            "wg": np.ascontiguousarray(wgc),
            "wrest": wrestc,
            "s": prep["SS"],
        })

    trace = bool(os.environ.get("BASS_TRACE"))
    if trace:
        bass_utils.upload_artifacts = lambda d: "local://skipped"
    try:
        res = bass_utils.run_bass_kernel_spmd(nc, in_maps,
                                              core_ids=list(range(NCORE)))
    except Exception:
        # transient NRT device errors have been observed; one retry
        res = bass_utils.run_bass_kernel_spmd(nc, in_maps,
                                              core_ids=list(range(NCORE)))
    if trace:
        LAST_EXEC_NS = res.exec_time_ns
        if res.exec_time_ns is not None:
            print(f"HW exec time: {res.exec_time_ns} ns")

    out = np.zeros((B, C), dtype=np.float32)
    for c in range(NCORE):
        o2 = res.results[c]["logits"]                          # [128, 32768]
        r0 = c * BC
        for t in range(NT):
            for j in range(NCH):
                col = (t * NCH + j) * 512
                strip = o2[0:ws[j], col:col + 512]             # [w_j, 512]
                out[r0 + t * 512:r0 + (t + 1) * 512, bins[j]] = \
                    strip.T.astype(np.float32)
    return out
